# revision 42
# baseline (speedup 1.0000x reference)
"""Trainium2 Bass kernel for nn_DistMaps: per-image Gaussian click maps.

out[b, c, i, j] = max over valid points p of channel c of
    exp(-((i - px_p)^2 + (j - py_p)^2) / r_p^2),  init 0.

Shapes (hardcoded): x [8, 3, 512, 512] (UNUSED by the math - only its batch
dim matters), coords [8, 48, 4] (px, py, order, radius); points 0..23 ->
channel 0, 24..47 -> channel 1. Output [8, 2, 512, 512] float32.

Strategy: data-parallel across 8 NeuronCores (1 batch image per core).
The Gaussian is separable: exp(-d2/r^2) = exp(-(i-px)^2/r^2) *
exp(-(j-py)^2/r^2), so per point we build row/col 1-D factor tables
A[p, i], B[p, j] (invalid points forced to 0 via a -1e30 exp scale), then
materialize each outer product A[p,:]^T B[p,:] with a K=1 TensorEngine
matmul into PSUM and max-accumulate on the VectorEngine.
"""

import contextlib
import ctypes
import sys
import types

import numpy as np

import concourse.bass as bass
import concourse.mybir as mybir
from concourse import bass_utils
from concourse import tile
from concourse.bass_utils import run_bass_kernel_spmd
from concourse.tile import ScopedClock, TileContext

B, C, H, W, P2 = 8, 3, 512, 512, 48
HALF = P2 // 2  # 24 points per channel
N_CORES = 8
F32 = mybir.dt.float32
F32R = mybir.dt.float32r
I32 = mybir.dt.int32
AF = mybir.ActivationFunctionType
OP = mybir.AluOpType


# --------------------------------------------------------------------------
# Environment patches for this container (axon-tunneled TRN2):
#  1. This walrus build rejects >1 sem-wait on CTRL-class instructions; the
#     stock Tile tail drain carries one wait per outstanding semaphore.
#     Split them one-per-NOP ahead of a bare drain.
#  2. NTFF profiling hook: antenv.axon_hooks is absent in this image; provide
#     it via ctypes against libaxon_pjrt.so so trace=True works.
#  3. Artifact upload has no bucket credentials here; make it a no-op.
# --------------------------------------------------------------------------
def _patched_drain_and_barrier(self, tick_clock, wait_clock):
    nc = self.nc
    probe = nc.sync.nop(nofuse=True, hint="tail_wait_probe")
    wait_clock.add_sem_waits(probe.ins, ScopedClock({None: tick_clock.global_clock}))
    si = probe.ins.sync_info
    waits = list(si.on_wait or []) if si is not None else []
    if len(waits) > 1:
        si.on_wait = waits[:1]
        for w in waits[1:]:
            n = nc.sync.nop(nofuse=True, hint="tail_wait_nop")
            n.ins.sync_info = mybir.SyncInfo(on_wait=[w], on_update=[])
    nc.sync.drain()
    nc.all_engine_barrier()
    assert self.sems is not None
    popped = nc._tile_sem_poison_stack.pop()
    assert popped is self._sem_poison
    nc.clear_and_free_semaphores(list(self.sems.allocated().values()))
    nc.all_engine_barrier()


TileContext._drain_and_barrier = _patched_drain_and_barrier


def _make_ntff_hook(so_path="/opt/axon/libaxon_pjrt.so"):
    try:
        lib = ctypes.CDLL(so_path)
    except OSError:
        return None
    if not hasattr(lib, "axon_start_nrt_profile"):
        return None
    lib.axon_start_nrt_profile.argtypes = [
        ctypes.POINTER(ctypes.c_int64),
        ctypes.c_size_t,
    ]
    lib.axon_start_nrt_profile.restype = ctypes.c_int64
    lib.axon_stop_nrt_profile.argtypes = [ctypes.c_char_p]
    lib.axon_stop_nrt_profile.restype = ctypes.c_int64

    @contextlib.contextmanager
    def _hook(output_dir, device_ids):
        import jax

        jax.devices()
        if device_ids:
            ids = (ctypes.c_int64 * len(device_ids))(*device_ids)
            rc = lib.axon_start_nrt_profile(ids, len(device_ids))
        else:
            rc = lib.axon_start_nrt_profile(None, 0)
        if rc != 0:
            raise RuntimeError(f"axon_start_nrt_profile rc={rc}")
        try:
            yield
        finally:
            n = lib.axon_stop_nrt_profile(str(output_dir).encode())
            print(f"ntff profile: {n} file(s) -> {output_dir}", file=sys.stderr)

    return _hook


if "antenv.axon_hooks" not in sys.modules:
    _m = types.ModuleType("antenv.axon_hooks")
    _the_hook = _make_ntff_hook()
    _m.get_axon_ntff_profile_hook = lambda: _the_hook
    _m.set_axon_ntff_profile_hook = lambda h: None
    sys.modules["antenv.axon_hooks"] = _m

bass_utils.upload_artifacts = lambda tmpdir: f"file://{tmpdir}"

_WSPLIT_N = [0]


def _split_multi_waits(nc: bass.Bass) -> int:
    """This walrus build accepts at most one sem-wait per instruction (and
    none on Drain). Hoist extra waits onto same-engine NOPs placed just
    before the instruction - engine stalls at the NOP instead, semantics
    unchanged."""
    n_split = 0
    for f in nc.m.functions:
        for blk in f.blocks:
            insts = blk.instructions
            new_list = []
            for inst in insts:
                si = inst.sync_info
                waits = list(si.on_wait) if (si is not None and si.on_wait) else []
                keep = 0 if inst.opcode == "Drain" else 1
                if len(waits) > keep:
                    moved = waits[: len(waits) - keep]
                    for w in moved:
                        _WSPLIT_N[0] += 1
                        nop = mybir.InstNoOp(
                            name=f"wsplit-{_WSPLIT_N[0]}", ins=[], outs=[]
                        )
                        nop.engine = inst.engine
                        nop.sync_info = mybir.SyncInfo(on_wait=[w], on_update=[])
                        new_list.append(nop)
                        n_split += 1
                    si.on_wait = waits[len(waits) - keep :]
                new_list.append(inst)
            if len(new_list) != len(insts):
                insts[:] = new_list
    return n_split


# --------------------------------------------------------------------------
# Kernel build
# --------------------------------------------------------------------------
BF16 = mybir.dt.bfloat16
# Per channel: the first TS_PTS points get their outer products from the
# vector engine (tensor_scalar from a partition-broadcast B table); the rest
# go through the tensor engine (K=1 matmul -> PSUM) with the scalar engine
# converting PSUM f32 -> SBUF bf16. The vector engine does every max
# accumulation as bf16 tensor_tensor (2x mode). This balances DVE/ACT/PE
# at roughly equal busy time; gpsimd can't run elementwise ops on this
# toolchain and PSUM data can only be read by ACT/DVE.
TS_PTS = 6


def build_nc() -> bass.Bass:
    nc = bass.Bass()
    coords = nc.declare_dram_parameter("coords", [P2, 4], F32, isOutput=False)
    out = nc.declare_dram_parameter("out", [2, H, W], F32, isOutput=True)
    bstage = nc.dram_tensor("bstage", [P2, W], BF16)  # DRAM bounce for bcast
    pstage = nc.dram_tensor("pstage", [2, P2], F32)  # px/s bounce for bcast

    with TileContext(nc) as tc:
        with (
            tc.tile_pool(name="tables", bufs=1) as tpool,
            tc.tile_pool(name="acc", bufs=1) as apool,
            tc.tile_pool(name="prod", bufs=12) as prpool,
            tc.tile_pool(name="fout", bufs=2) as fpool,
            tc.tile_pool(name="psum", bufs=4, space="PSUM") as ppool,
        ):
            # ---- input-independent setup first (iota, transpose identity)
            # so the serial coords-dependent prefix is as short as possible
            idxf = tpool.tile([P2, W], F32)
            # values 0..511 are exact in f32
            nc.gpsimd.iota(idxf[:], pattern=[[1, W]], base=0,
                           channel_multiplier=0,
                           allow_small_or_imprecise_dtypes=True)
            ones48 = tpool.tile([P2, P2], BF16)
            ident48 = tpool.tile([P2, P2], BF16)
            nc.gpsimd.memset(ones48[:], 1.0)
            nc.gpsimd.affine_select(
                ident48[:], ones48[:], pattern=[[-1, P2]],
                compare_op=OP.is_equal, fill=0.0, base=0, channel_multiplier=1,
            )

            # ---- per-point scalars (points on partitions) ----
            ct = tpool.tile([P2, 4], F32)
            nc.sync.dma_start(ct[:], coords[:])
            px = ct[:, 0:1]
            py = ct[:, 1:2]
            r = ct[:, 3:4]

            vx = tpool.tile([P2, 1], F32)
            vy = tpool.tile([P2, 1], F32)
            v = tpool.tile([P2, 1], F32)
            nc.vector.tensor_scalar(vx[:], px, 0.0, None, OP.is_ge)
            nc.vector.tensor_scalar(vy[:], py, 0.0, None, OP.is_ge)
            nc.vector.tensor_tensor(v[:], vx[:], vy[:], OP.mult)

            r2 = tpool.tile([P2, 1], F32)
            inv = tpool.tile([P2, 1], F32)
            nc.vector.tensor_tensor(r2[:], r, r, OP.mult)
            nc.vector.reciprocal(inv[:], r2[:])

            # s = valid ? -1/r^2 : ~-1e30   (kills invalid points: exp -> 0)
            t1 = tpool.tile([P2, 1], F32)
            vm1 = tpool.tile([P2, 1], F32)
            s = tpool.tile([P2, 1], F32)
            nc.vector.tensor_tensor(t1[:], inv[:], v[:], OP.mult)
            nc.vector.tensor_scalar(vm1[:], v[:], -1.0, None, OP.add)
            nc.vector.scalar_tensor_tensor(
                s[:], vm1[:], 1.0e30, t1[:], OP.mult, OP.subtract
            )

            # ---- 1-D tables: tab{A,B}[p, i] = exp(s_p * (i - p{x,y}_p)^2),
            # bf16, points on partitions ----
            dA = tpool.tile([P2, H], F32)
            dB = tpool.tile([P2, W], F32)
            tabA = tpool.tile([P2, H], BF16)
            tabB = tpool.tile([P2, W], BF16)
            nc.vector.tensor_scalar(dA[:], idxf[:], px, None, OP.subtract)
            nc.vector.scalar_tensor_tensor(dA[:], dA[:], s[:], dA[:], OP.mult, OP.mult)
            nc.scalar.activation(tabA[:], dA[:], AF.Exp)
            nc.vector.tensor_scalar(dB[:], idxf[:], py, None, OP.subtract)
            nc.vector.scalar_tensor_tensor(dB[:], dB[:], s[:], dB[:], OP.mult, OP.mult)
            nc.scalar.activation(tabB[:], dB[:], AF.Exp)

            # ---- flat copies for the PE path (operand base partition must
            # be 0/32): channel c tables at partition 32c ----
            Aflat = tpool.tile([64, HALF * H], BF16)
            Bflat = tpool.tile([64, HALF * W], BF16)
            for c in range(2):
                bp = c * 32
                nc.sync.dma_start(
                    Aflat[bp : bp + 1, :].rearrange("o (p j) -> o p j", j=H),
                    tabA[c * HALF : (c + 1) * HALF, :],
                )
                nc.sync.dma_start(
                    Bflat[bp : bp + 1, :].rearrange("o (p j) -> o p j", j=W),
                    tabB[c * HALF : (c + 1) * HALF, :],
                )

            # ---- broadcast the TS-path points' B rows across partitions
            # (via DRAM bounce; SBUF sources can't have 0-step partition
            # dims, DRAM sources can) ----
            nc.sync.dma_start(bstage[0:TS_PTS, :], tabB[0:TS_PTS, :])
            nc.sync.dma_start(
                bstage[TS_PTS : 2 * TS_PTS, :],
                tabB[HALF : HALF + TS_PTS, :],
            )
            Bb = tpool.tile([128, 2 * TS_PTS, W], BF16)
            nc.sync.dma_start(
                Bb[:], bstage[0 : 2 * TS_PTS, :].partition_broadcast(128)
            )

            # ---- A_T via PE transpose: at[i, t, p] = tabA[p, t*128+i],
            # f32 (the per-partition scalar operand on the TS path) ----
            at = tpool.tile([128, 4, P2], F32)
            # borrow one main-psum slot for the transposes (released after
            # the copy, before the matmul pipeline needs all four)
            atp = ppool.tile([128, 4 * P2], BF16, tag="ps", name="atp")
            for t in range(4):
                nc.tensor.transpose(
                    atp[:, t * P2 : (t + 1) * P2],
                    tabA[:, t * 128 : (t + 1) * 128],
                    ident48[:],
                )
            nc.scalar.copy(at[:], atp[:])

            # ---- main accumulation ----
            # The PE/ACT-fed main chain (points TS_PTS..23) and a decoupled
            # DVE-only side tree over the first TS_PTS points (fused
            # mult+max STT into tsacc); one TT merges the side tree into acc
            # at the end. Decoupling lets the vector engine run the side
            # tree whenever the main chain leaves it idle.
            accs = [
                apool.tile([128, 4, W], BF16, tag=f"acc{c}", name=f"acc{c}")
                for c in range(2)
            ]
            tsaccs = [
                apool.tile([128, 4, W], BF16, tag=f"tsacc{c}", name=f"tsacc{c}")
                for c in range(2)
            ]

            def emit_pe(c, p, dest):
                # dest: bf16 [128, 4, W] tile the product lands in
                bp = c * 32
                for h in range(2):
                    ps = ppool.tile([128, 2, W], F32, tag="ps", name="ps")
                    for tt in range(2):
                        t = h * 2 + tt
                        nc.tensor.matmul(
                            ps[:, tt, :],
                            Aflat[bp : bp + 1,
                                  p * H + t * 128 : p * H + (t + 1) * 128],
                            Bflat[bp : bp + 1, p * W : (p + 1) * W],
                            start=True,
                            stop=True,
                        )
                    nc.scalar.copy(dest[:, h * 2 : h * 2 + 2, :], ps[:])

            ts_emitted = [0, 0]

            def emit_ts_point(c):
                ki = ts_emitted[c]
                ts_emitted[c] += 1
                g = c * HALF + ki
                tsacc = tsaccs[c]
                # products via tensor_scalar (4x mode) then one 2x-mode
                # tensor_tensor max - faster than the 1x-only fused STT
                dest = tsacc if ki == 0 else prpool.tile(
                    [128, 4, W], BF16, tag="prod", name="tsprod")
                for t in range(4):
                    nc.vector.tensor_scalar(
                        dest[:, t, :], Bb[:, c * TS_PTS + ki, :],
                        at[:, t, g : g + 1], None, OP.mult,
                    )
                if ki > 0:
                    nc.vector.tensor_tensor(tsacc[:], dest[:], tsacc[:],
                                            OP.max)

            for slot in range(HALF - TS_PTS):
                for c in range(2):
                    p = TS_PTS + slot
                    if slot == 0:
                        emit_pe(c, p, accs[c])  # ACT writes acc: free init
                        continue
                    prod = prpool.tile([128, 4, W], BF16, tag="prod",
                                       name="prod")
                    emit_pe(c, p, prod)
                    nc.vector.tensor_tensor(accs[c][:], prod[:], accs[c][:],
                                            OP.max)
                if slot % 2 == 1:
                    for c in range(2):
                        if ts_emitted[c] < TS_PTS:
                            emit_ts_point(c)
            while ts_emitted[0] < TS_PTS or ts_emitted[1] < TS_PTS:
                for c in range(2):
                    if ts_emitted[c] < TS_PTS:
                        emit_ts_point(c)
            # ---- final merge of the side tree writes f32 directly (the
            # widening TT replaces a separate scalar-engine convert), in
            # half-tiles so each DMA starts as soon as its half merges ----
            for c in range(2):
                f32o = fpool.tile([128, 4, W], F32, tag="f32o", name="f32o")
                outv = out[c].rearrange("(t p) j -> p t j", p=128)
                for hh in range(2):
                    sl = slice(2 * hh, 2 * hh + 2)
                    nc.vector.tensor_tensor(
                        f32o[:, sl, :], tsaccs[c][:, sl, :],
                        accs[c][:, sl, :], OP.max,
                    )
                    nc.sync.dma_start(outv[:, sl, :], f32o[:, sl, :])
    _split_multi_waits(nc)
    return nc


_NC_CACHE: bass.Bass | None = None


def _get_nc() -> bass.Bass:
    global _NC_CACHE
    if _NC_CACHE is None:
        _NC_CACHE = build_nc()
    return _NC_CACHE


def run(coords_full: np.ndarray, trace: bool = False):
    """coords_full: [8, 48, 4] float32. Returns ([8,2,H,W] float32, results)."""
    nc = _get_nc()
    in_maps = [
        {"coords": np.ascontiguousarray(coords_full[b], dtype=np.float32)}
        for b in range(B)
    ]
    res = run_bass_kernel_spmd(nc, in_maps, list(range(N_CORES)), trace=trace)
    outs = np.stack([res.results[b]["out"] for b in range(B)], axis=0)
    return outs.astype(np.float32), res


def kernel(x: np.ndarray, coords: np.ndarray) -> np.ndarray:
    out, _ = run(np.asarray(coords), trace=False)
    return out


# revision 43
# speedup vs baseline: 1.0095x; 1.0095x over previous
"""Trainium2 Bass kernel for nn_DistMaps: per-image Gaussian click maps.

out[b, c, i, j] = max over valid points p of channel c of
    exp(-((i - px_p)^2 + (j - py_p)^2) / r_p^2),  init 0.

Shapes (hardcoded): x [8, 3, 512, 512] (UNUSED by the math - only its batch
dim matters), coords [8, 48, 4] (px, py, order, radius); points 0..23 ->
channel 0, 24..47 -> channel 1. Output [8, 2, 512, 512] float32.

Strategy: data-parallel across 8 NeuronCores (1 batch image per core).
The Gaussian is separable: exp(-d2/r^2) = exp(-(i-px)^2/r^2) *
exp(-(j-py)^2/r^2), so per point we build row/col 1-D factor tables
A[p, i], B[p, j] (invalid points forced to 0 via a -1e30 exp scale), then
materialize each outer product A[p,:]^T B[p,:] with a K=1 TensorEngine
matmul into PSUM and max-accumulate on the VectorEngine.
"""

import contextlib
import ctypes
import sys
import types

import numpy as np

import concourse.bass as bass
import concourse.mybir as mybir
from concourse import bass_utils
from concourse import tile
from concourse.bass_utils import run_bass_kernel_spmd
from concourse.tile import ScopedClock, TileContext

B, C, H, W, P2 = 8, 3, 512, 512, 48
HALF = P2 // 2  # 24 points per channel
N_CORES = 8
F32 = mybir.dt.float32
F32R = mybir.dt.float32r
I32 = mybir.dt.int32
AF = mybir.ActivationFunctionType
OP = mybir.AluOpType


# --------------------------------------------------------------------------
# Environment patches for this container (axon-tunneled TRN2):
#  1. This walrus build rejects >1 sem-wait on CTRL-class instructions; the
#     stock Tile tail drain carries one wait per outstanding semaphore.
#     Split them one-per-NOP ahead of a bare drain.
#  2. NTFF profiling hook: antenv.axon_hooks is absent in this image; provide
#     it via ctypes against libaxon_pjrt.so so trace=True works.
#  3. Artifact upload has no bucket credentials here; make it a no-op.
# --------------------------------------------------------------------------
def _patched_drain_and_barrier(self, tick_clock, wait_clock):
    nc = self.nc
    probe = nc.sync.nop(nofuse=True, hint="tail_wait_probe")
    wait_clock.add_sem_waits(probe.ins, ScopedClock({None: tick_clock.global_clock}))
    si = probe.ins.sync_info
    waits = list(si.on_wait or []) if si is not None else []
    if len(waits) > 1:
        si.on_wait = waits[:1]
        for w in waits[1:]:
            n = nc.sync.nop(nofuse=True, hint="tail_wait_nop")
            n.ins.sync_info = mybir.SyncInfo(on_wait=[w], on_update=[])
    nc.sync.drain()
    nc.all_engine_barrier()
    assert self.sems is not None
    popped = nc._tile_sem_poison_stack.pop()
    assert popped is self._sem_poison
    nc.clear_and_free_semaphores(list(self.sems.allocated().values()))
    nc.all_engine_barrier()


TileContext._drain_and_barrier = _patched_drain_and_barrier


def _make_ntff_hook(so_path="/opt/axon/libaxon_pjrt.so"):
    try:
        lib = ctypes.CDLL(so_path)
    except OSError:
        return None
    if not hasattr(lib, "axon_start_nrt_profile"):
        return None
    lib.axon_start_nrt_profile.argtypes = [
        ctypes.POINTER(ctypes.c_int64),
        ctypes.c_size_t,
    ]
    lib.axon_start_nrt_profile.restype = ctypes.c_int64
    lib.axon_stop_nrt_profile.argtypes = [ctypes.c_char_p]
    lib.axon_stop_nrt_profile.restype = ctypes.c_int64

    @contextlib.contextmanager
    def _hook(output_dir, device_ids):
        import jax

        jax.devices()
        if device_ids:
            ids = (ctypes.c_int64 * len(device_ids))(*device_ids)
            rc = lib.axon_start_nrt_profile(ids, len(device_ids))
        else:
            rc = lib.axon_start_nrt_profile(None, 0)
        if rc != 0:
            raise RuntimeError(f"axon_start_nrt_profile rc={rc}")
        try:
            yield
        finally:
            n = lib.axon_stop_nrt_profile(str(output_dir).encode())
            print(f"ntff profile: {n} file(s) -> {output_dir}", file=sys.stderr)

    return _hook


if "antenv.axon_hooks" not in sys.modules:
    _m = types.ModuleType("antenv.axon_hooks")
    _the_hook = _make_ntff_hook()
    _m.get_axon_ntff_profile_hook = lambda: _the_hook
    _m.set_axon_ntff_profile_hook = lambda h: None
    sys.modules["antenv.axon_hooks"] = _m

bass_utils.upload_artifacts = lambda tmpdir: f"file://{tmpdir}"

_WSPLIT_N = [0]


def _split_multi_waits(nc: bass.Bass) -> int:
    """This walrus build accepts at most one sem-wait per instruction (and
    none on Drain). Hoist extra waits onto same-engine NOPs placed just
    before the instruction - engine stalls at the NOP instead, semantics
    unchanged."""
    n_split = 0
    for f in nc.m.functions:
        for blk in f.blocks:
            insts = blk.instructions
            new_list = []
            for inst in insts:
                si = inst.sync_info
                waits = list(si.on_wait) if (si is not None and si.on_wait) else []
                keep = 0 if inst.opcode == "Drain" else 1
                if len(waits) > keep:
                    moved = waits[: len(waits) - keep]
                    for w in moved:
                        _WSPLIT_N[0] += 1
                        nop = mybir.InstNoOp(
                            name=f"wsplit-{_WSPLIT_N[0]}", ins=[], outs=[]
                        )
                        nop.engine = inst.engine
                        nop.sync_info = mybir.SyncInfo(on_wait=[w], on_update=[])
                        new_list.append(nop)
                        n_split += 1
                    si.on_wait = waits[len(waits) - keep :]
                new_list.append(inst)
            if len(new_list) != len(insts):
                insts[:] = new_list
    return n_split


# --------------------------------------------------------------------------
# Kernel build
# --------------------------------------------------------------------------
BF16 = mybir.dt.bfloat16
# Per channel: the first TS_PTS points get their outer products from the
# vector engine (tensor_scalar from a partition-broadcast B table); the rest
# go through the tensor engine (K=1 matmul -> PSUM) with the scalar engine
# converting PSUM f32 -> SBUF bf16. The vector engine does every max
# accumulation as bf16 tensor_tensor (2x mode). This balances DVE/ACT/PE
# at roughly equal busy time; gpsimd can't run elementwise ops on this
# toolchain and PSUM data can only be read by ACT/DVE.
TS_PTS = 7


def build_nc() -> bass.Bass:
    nc = bass.Bass()
    coords = nc.declare_dram_parameter("coords", [P2, 4], F32, isOutput=False)
    out = nc.declare_dram_parameter("out", [2, H, W], F32, isOutput=True)
    bstage = nc.dram_tensor("bstage", [P2, W], BF16)  # DRAM bounce for bcast
    pstage = nc.dram_tensor("pstage", [2, P2], F32)  # px/s bounce for bcast

    with TileContext(nc) as tc:
        with (
            tc.tile_pool(name="tables", bufs=1) as tpool,
            tc.tile_pool(name="acc", bufs=1) as apool,
            tc.tile_pool(name="prod", bufs=12) as prpool,
            tc.tile_pool(name="fout", bufs=2) as fpool,
            tc.tile_pool(name="psum", bufs=4, space="PSUM") as ppool,
        ):
            # ---- input-independent setup first (iota, transpose identity)
            # so the serial coords-dependent prefix is as short as possible
            idxf = tpool.tile([P2, W], F32)
            # values 0..511 are exact in f32
            nc.gpsimd.iota(idxf[:], pattern=[[1, W]], base=0,
                           channel_multiplier=0,
                           allow_small_or_imprecise_dtypes=True)
            ones48 = tpool.tile([P2, P2], BF16)
            ident48 = tpool.tile([P2, P2], BF16)
            nc.gpsimd.memset(ones48[:], 1.0)
            nc.gpsimd.affine_select(
                ident48[:], ones48[:], pattern=[[-1, P2]],
                compare_op=OP.is_equal, fill=0.0, base=0, channel_multiplier=1,
            )

            # ---- per-point scalars (points on partitions) ----
            ct = tpool.tile([P2, 4], F32)
            nc.sync.dma_start(ct[:], coords[:])
            px = ct[:, 0:1]
            py = ct[:, 1:2]
            r = ct[:, 3:4]

            vx = tpool.tile([P2, 1], F32)
            vy = tpool.tile([P2, 1], F32)
            v = tpool.tile([P2, 1], F32)
            nc.vector.tensor_scalar(vx[:], px, 0.0, None, OP.is_ge)
            nc.vector.tensor_scalar(vy[:], py, 0.0, None, OP.is_ge)
            nc.vector.tensor_tensor(v[:], vx[:], vy[:], OP.mult)

            r2 = tpool.tile([P2, 1], F32)
            inv = tpool.tile([P2, 1], F32)
            nc.vector.tensor_tensor(r2[:], r, r, OP.mult)
            nc.vector.reciprocal(inv[:], r2[:])

            # s = valid ? -1/r^2 : ~-1e30   (kills invalid points: exp -> 0)
            t1 = tpool.tile([P2, 1], F32)
            vm1 = tpool.tile([P2, 1], F32)
            s = tpool.tile([P2, 1], F32)
            nc.vector.tensor_tensor(t1[:], inv[:], v[:], OP.mult)
            nc.vector.tensor_scalar(vm1[:], v[:], -1.0, None, OP.add)
            nc.vector.scalar_tensor_tensor(
                s[:], vm1[:], 1.0e30, t1[:], OP.mult, OP.subtract
            )

            # ---- 1-D tables: tab{A,B}[p, i] = exp(s_p * (i - p{x,y}_p)^2),
            # bf16, points on partitions ----
            dA = tpool.tile([P2, H], F32)
            dB = tpool.tile([P2, W], F32)
            tabA = tpool.tile([P2, H], BF16)
            tabB = tpool.tile([P2, W], BF16)
            nc.vector.tensor_scalar(dA[:], idxf[:], px, None, OP.subtract)
            nc.vector.scalar_tensor_tensor(dA[:], dA[:], s[:], dA[:], OP.mult, OP.mult)
            nc.scalar.activation(tabA[:], dA[:], AF.Exp)
            nc.vector.tensor_scalar(dB[:], idxf[:], py, None, OP.subtract)
            nc.vector.scalar_tensor_tensor(dB[:], dB[:], s[:], dB[:], OP.mult, OP.mult)
            nc.scalar.activation(tabB[:], dB[:], AF.Exp)

            # ---- flat copies for the PE path (operand base partition must
            # be 0/32): channel c tables at partition 32c ----
            Aflat = tpool.tile([64, HALF * H], BF16)
            Bflat = tpool.tile([64, HALF * W], BF16)
            for c in range(2):
                bp = c * 32
                nc.sync.dma_start(
                    Aflat[bp : bp + 1, :].rearrange("o (p j) -> o p j", j=H),
                    tabA[c * HALF : (c + 1) * HALF, :],
                )
                nc.sync.dma_start(
                    Bflat[bp : bp + 1, :].rearrange("o (p j) -> o p j", j=W),
                    tabB[c * HALF : (c + 1) * HALF, :],
                )

            # ---- broadcast the TS-path points' B rows across partitions
            # (via DRAM bounce; SBUF sources can't have 0-step partition
            # dims, DRAM sources can) ----
            nc.sync.dma_start(bstage[0:TS_PTS, :], tabB[0:TS_PTS, :])
            nc.sync.dma_start(
                bstage[TS_PTS : 2 * TS_PTS, :],
                tabB[HALF : HALF + TS_PTS, :],
            )
            Bb = tpool.tile([128, 2 * TS_PTS, W], BF16)
            nc.sync.dma_start(
                Bb[:], bstage[0 : 2 * TS_PTS, :].partition_broadcast(128)
            )

            # ---- A_T via PE transpose: at[i, t, p] = tabA[p, t*128+i],
            # f32 (the per-partition scalar operand on the TS path) ----
            at = tpool.tile([128, 4, P2], F32)
            # borrow one main-psum slot for the transposes (released after
            # the copy, before the matmul pipeline needs all four)
            atp = ppool.tile([128, 4 * P2], BF16, tag="ps", name="atp")
            for t in range(4):
                nc.tensor.transpose(
                    atp[:, t * P2 : (t + 1) * P2],
                    tabA[:, t * 128 : (t + 1) * 128],
                    ident48[:],
                )
            nc.scalar.copy(at[:], atp[:])

            # ---- main accumulation ----
            # The PE/ACT-fed main chain (points TS_PTS..23) and a decoupled
            # DVE-only side tree over the first TS_PTS points (fused
            # mult+max STT into tsacc); one TT merges the side tree into acc
            # at the end. Decoupling lets the vector engine run the side
            # tree whenever the main chain leaves it idle.
            accs = [
                apool.tile([128, 4, W], BF16, tag=f"acc{c}", name=f"acc{c}")
                for c in range(2)
            ]
            tsaccs = [
                apool.tile([128, 4, W], BF16, tag=f"tsacc{c}", name=f"tsacc{c}")
                for c in range(2)
            ]

            def emit_pe(c, p, dest):
                # dest: bf16 [128, 4, W] tile the product lands in
                bp = c * 32
                for h in range(2):
                    ps = ppool.tile([128, 2, W], F32, tag="ps", name="ps")
                    for tt in range(2):
                        t = h * 2 + tt
                        nc.tensor.matmul(
                            ps[:, tt, :],
                            Aflat[bp : bp + 1,
                                  p * H + t * 128 : p * H + (t + 1) * 128],
                            Bflat[bp : bp + 1, p * W : (p + 1) * W],
                            start=True,
                            stop=True,
                        )
                    nc.scalar.copy(dest[:, h * 2 : h * 2 + 2, :], ps[:])

            ts_emitted = [0, 0]

            def emit_ts_point(c):
                ki = ts_emitted[c]
                ts_emitted[c] += 1
                g = c * HALF + ki
                tsacc = tsaccs[c]
                # products via tensor_scalar (4x mode) then one 2x-mode
                # tensor_tensor max - faster than the 1x-only fused STT
                dest = tsacc if ki == 0 else prpool.tile(
                    [128, 4, W], BF16, tag="prod", name="tsprod")
                for t in range(4):
                    nc.vector.tensor_scalar(
                        dest[:, t, :], Bb[:, c * TS_PTS + ki, :],
                        at[:, t, g : g + 1], None, OP.mult,
                    )
                if ki > 0:
                    nc.vector.tensor_tensor(tsacc[:], dest[:], tsacc[:],
                                            OP.max)

            for slot in range(HALF - TS_PTS):
                for c in range(2):
                    p = TS_PTS + slot
                    if slot == 0:
                        emit_pe(c, p, accs[c])  # ACT writes acc: free init
                        continue
                    prod = prpool.tile([128, 4, W], BF16, tag="prod",
                                       name="prod")
                    emit_pe(c, p, prod)
                    nc.vector.tensor_tensor(accs[c][:], prod[:], accs[c][:],
                                            OP.max)
                if slot % 2 == 1:
                    for c in range(2):
                        if ts_emitted[c] < TS_PTS:
                            emit_ts_point(c)
            while ts_emitted[0] < TS_PTS or ts_emitted[1] < TS_PTS:
                for c in range(2):
                    if ts_emitted[c] < TS_PTS:
                        emit_ts_point(c)
            # ---- final merge of the side tree writes f32 directly (the
            # widening TT replaces a separate scalar-engine convert), in
            # half-tiles so each DMA starts as soon as its half merges ----
            for c in range(2):
                f32o = fpool.tile([128, 4, W], F32, tag="f32o", name="f32o")
                outv = out[c].rearrange("(t p) j -> p t j", p=128)
                for hh in range(2):
                    sl = slice(2 * hh, 2 * hh + 2)
                    nc.vector.tensor_tensor(
                        f32o[:, sl, :], tsaccs[c][:, sl, :],
                        accs[c][:, sl, :], OP.max,
                    )
                    nc.sync.dma_start(outv[:, sl, :], f32o[:, sl, :])
    _split_multi_waits(nc)
    return nc


_NC_CACHE: bass.Bass | None = None


def _get_nc() -> bass.Bass:
    global _NC_CACHE
    if _NC_CACHE is None:
        _NC_CACHE = build_nc()
    return _NC_CACHE


def run(coords_full: np.ndarray, trace: bool = False):
    """coords_full: [8, 48, 4] float32. Returns ([8,2,H,W] float32, results)."""
    nc = _get_nc()
    in_maps = [
        {"coords": np.ascontiguousarray(coords_full[b], dtype=np.float32)}
        for b in range(B)
    ]
    res = run_bass_kernel_spmd(nc, in_maps, list(range(N_CORES)), trace=trace)
    outs = np.stack([res.results[b]["out"] for b in range(B)], axis=0)
    return outs.astype(np.float32), res


def kernel(x: np.ndarray, coords: np.ndarray) -> np.ndarray:
    out, _ = run(np.asarray(coords), trace=False)
    return out
